# revision 34
# baseline (speedup 1.0000x reference)
"""Contrastive pairwise-margin loss on 8 Trainium2 NeuronCores.

loss = sum_{i,j} [ R_ij * d_ij + (1-R_ij) * relu(0.5 - d_ij) ] / (N*(N-1)*2)
with d_ij = ||x_i - x_j||^2 and R_ij = [t_i == t_j].

Decomposition (host rows sorted by class):
  loss_sum = sum_{i!=j} relu(m - d_ij)  +  sum_{same-class, i!=j} [d - relu(m-d)]
The second term is exact fp64 on the host (O(sum n_c^2 * D), tiny). The device
computes the uniform all-pairs relu sum over the upper-triangle blocks:
  relu(m - d_ij) = 2 * relu(a_ij),  a_ij = g_ij + (m - sq_i - sq_j)/2
Per [128,<=512] tile: fp8 DoubleRow gram matmuls accumulate g into PSUM; the
row part of the bias, B_i = m/2 - sq_i/2 - mean(sq)/2, is applied by the
relu+row-sum op itself (per-partition scalar operand) — no extra matmuls.
The column part (sq_j - mean(sq))/2 is dropped on device; since relu terms
are nonnegative, any false positive would show up as the device total
exceeding the exactly-emulated diagonal term, which the host checks, falling
back to an exact fp64 recompute if it ever failed (it cannot for data with
pairwise distances >> margin).  relu+row-sum is one fused op per PSUM group,
split between VectorE (tensor_scalar add-bias/max-0 + accum) and ScalarE
(activation Relu + bias + accum_out) by a static cost balance.  Groups mixing
row-chunks (diagonal strict-upper slices, packed 128x128 diagonal squares)
use the elementwise max of the chunks' bias vectors.  Host: everything x4
(2 from relu(2a)=2relu(a), 2 from block symmetry), x2 for the diagonal
squares (ordered pairs counted both ways), minus the exactly-emulated device
diagonal entries, plus the exact same-class term.
"""

import os
import sys

for _p in ("/opt/trn_rl_repo", "/root/.axon_site/_ro/trn_rl_repo"):
    if os.path.isdir(_p) and _p not in sys.path:
        sys.path.insert(0, _p)

from contextlib import ExitStack

import ml_dtypes
import numpy as np

import concourse.bass as bass  # noqa: F401
import concourse.mybir as mybir
from concourse import bacc, bass_utils
from concourse.tile import TileContext

FP8 = ml_dtypes.float8_e4m3
MARGIN = 0.5
N = 8192
D = 256
P = 128
BLK = 512
NBLK = N // BLK        # 16 stripes
NCORES = 8
NBLOCKS = 17           # blocks per core (2 diag + 15 off-diag)
NWARM = 6              # PE p-state warm-up matmuls

# operand slots: 0,1 = diag blocks (lhs==rhs); off-diag block i (pos 2..16)
# has lhs slot 2+2*(i-2), rhs slot 3+2*(i-2)  -> 2 + 15*2 = 32 slots
NSLOTS = 2 + 15 * 2

# DMA chunking in slot units (first chunks small so compute starts early)
SLOT_CHUNKS = [1, 1, 2, 2, 4, 6, 8, 8]
assert sum(SLOT_CHUNKS) == NSLOTS

# group table: (parts, width, wgt); parts = (block_pos, mi, rhs_off, w, pcol)
# diag block a: strict-upper slices split into TWO groups (one per engine, so
# both engines start on slot 0); diag block b merged into one 768 group; the
# eight 128x128 diagonal sub-squares of both diag blocks packed into one
# 1024-wide group (weight 2).
GROUPS = []
GROUPS.append(([(0, 0, 128, 384, 0)], 384, 4.0))
GROUPS.append(([(0, 1, 256, 256, 0), (0, 2, 384, 128, 256)], 384, 4.0))
GROUPS.append(([(0, mi, mi * P, P, mi * P) for mi in range(4)], 512, 2.0))
GROUPS.append(([(1, 0, 128, 384, 0), (1, 1, 256, 256, 384),
                (1, 2, 384, 128, 640)], 768, 4.0))
GROUPS.append(([(1, mi, mi * P, P, mi * P) for mi in range(4)], 512, 2.0))
# off-diagonal blocks: 11 blocks contribute one 1536-wide ACT group (wide
# ops amortize ScalarE's per-op tax) + one 512 DVE group; 4 blocks spread
# through the stream are all-DVE (4x512) so VectorE keeps pace.
_ALLD = {2, 6, 10, 14}
for _pos in range(2, NBLOCKS):
    if _pos in _ALLD:
        for _mi in range(4):
            GROUPS.append(([(_pos, _mi, 0, BLK, 0)], BLK, 4.0))
    else:
        GROUPS.append(([(_pos, _mi, 0, BLK, _mi * BLK)
                        for _mi in range(3)], 3 * BLK, 4.0))
        GROUPS.append(([(_pos, 3, 0, BLK, 0)], BLK, 4.0))
NGROUPS = len(GROUPS)
SQ_GRPS = [i for i, g in enumerate(GROUPS) if g[2] == 2.0]

# static engine split: strict alternation is globally balanced (widths are
# symmetric) and locally smooth, so neither engine starves on the PSUM ring
def _op_cost(width, eng):
    if eng == "dve":
        return (width + 120) * 1.0417
    return (width + 172) * 0.8333 + 187

def _assign_engines():
    # head: the slot0-only half of the diagonal squares (g2) keeps DVE busy
    # across the slot-1 arrival seam; off-diagonals alternate DVE-first
    head = ["dve", "act", "dve", "act", "dve"]
    tail = ["act" if GROUPS[5 + i][1] > 1024 else "dve"
            for i in range(len(GROUPS) - 5)]
    return head + tail

ENGINES = _assign_engines()
DVE_COLS = [i for i, e in enumerate(ENGINES) if e == "dve"]
ACC_COL = {}
for _i, _g in enumerate(DVE_COLS):
    ACC_COL[_g] = ("dve", _i)
_ACT_COLS = [i for i, e in enumerate(ENGINES) if e == "act"]
for _i, _g in enumerate(_ACT_COLS):
    ACC_COL[_g] = ("act", _i)
N_DVE = len(DVE_COLS)
N_ACT = len(_ACT_COLS)
W_DVE = np.array([GROUPS[g][2] for g in DVE_COLS])
W_ACT = np.array([GROUPS[g][2] for g in _ACT_COLS])

_CHUNK_OF_SLOT = {}
_c0 = 0
for _g, _n in enumerate(SLOT_CHUNKS):
    for _s in range(_c0, _c0 + _n):
        _CHUNK_OF_SLOT[_s] = (_g, _s - _c0)
    _c0 += _n

def _pos_slots(pos):
    """(lhs_slot, rhs_slot) for block position."""
    if pos < 2:
        return pos, pos
    return 2 + 2 * (pos - 2), 3 + 2 * (pos - 2)

_COMPILED = None
LAST_RESULTS = None


def _build_program():
    nc = bacc.Bacc("TRN2", target_bir_lowering=False, debug=False,
                   num_devices=NCORES)
    f8 = mybir.dt.float8e4
    f32 = mybir.dt.float32
    Alu = mybir.AluOpType
    Relu = mybir.ActivationFunctionType.Relu

    ops_d = nc.dram_tensor("ops", [P, NSLOTS * 2, BLK], f8,
                           kind="ExternalInput")
    bias_d = nc.dram_tensor("biast", [P, NGROUPS], f32, kind="ExternalInput")
    acc_d = nc.dram_tensor("acc", [P, NGROUPS], f32, kind="ExternalOutput")

    with TileContext(nc) as tc, ExitStack() as ctx:
        sb = ctx.enter_context(tc.tile_pool(name="sb", bufs=1))
        wpool = ctx.enter_context(tc.tile_pool(name="wpool", bufs=2))
        ppa = ctx.enter_context(tc.tile_pool(name="ppa", bufs=2, space="PSUM"))
        ppd = ctx.enter_context(tc.tile_pool(name="ppd", bufs=2, space="PSUM"))

        bias_t = sb.tile([P, NGROUPS], f32)
        acc_t = sb.tile([P, NGROUPS], f32)

        # input DMAs first, spread over three descriptor-generator paths so
        # they pipeline: slot chunk 0 via gpsimd SWDGE (cheapest first-byte),
        # the first bias columns via the DVE-issued HWDGE, the rest via sync.
        slots_g = []
        c0 = 0
        for g, nb in enumerate(SLOT_CHUNKS):
            bt = sb.tile([P, nb * 2, BLK], f8, tag=f"ops{g}")
            if g == 0:
                nc.sync.dma_start(bt[:], ops_d[:, 0:2, :])
                nc.gpsimd.dma_start(bias_t[:], bias_d[:])
            else:
                nc.sync.dma_start(bt[:], ops_d[:, c0 * 2:(c0 + nb) * 2, :])
            slots_g.append((c0, bt))
            c0 += nb

        # warm the ACT table set while DMAs ramp (hides LoadActFuncSet)
        warm = wpool.tile([P, 1], f32, tag="warm")
        nc.gpsimd.memset(warm[:], 0.0)
        nc.scalar.activation(warm[:], warm[:], Relu)

        # PE p-state warm-up: chain of cheap matmuls on a zeroed operand
        wop = wpool.tile([P, 2, 256], f8, tag="wop")
        nc.gpsimd.memset(wop[:], 0.0)
        wp = ppa.tile([P, 3 * BLK], f32, tag="pa")
        DR = mybir.MatmulPerfMode.DoubleRow
        for _ in range(NWARM):
            nc.tensor.matmul(wp[:, 0:256], wop[:, :, 0:P], wop[:],
                             start=True, stop=True, perf_mode=DR)

        def slot_ap(s):
            g, off = _CHUNK_OF_SLOT[s]
            bt = slots_g[g][1]
            return bt[:, off * 2:off * 2 + 2, :]   # [128, 2, 512]

        nbulk = NGROUPS - 8
        for grp, (parts, width, _wgt) in enumerate(GROUPS):
            if ENGINES[grp] == "act":
                p_t = ppa.tile([P, 3 * BLK], f32, tag="pa")
            else:
                p_t = ppd.tile([P, BLK], f32, tag="pd")
            for pos, mi, off, w, pcol in parts:
                ls, rs = _pos_slots(pos)
                lhs3 = slot_ap(ls)
                rhs3 = slot_ap(rs)
                sl = slice(pcol, pcol + w)
                nc.tensor.matmul(p_t[:, sl],
                                 lhs3[:, :, mi * P:(mi + 1) * P],
                                 rhs3[:, :, off:off + w],
                                 start=True, stop=True, perf_mode=DR)
            eng = ENGINES[grp]
            bslice = bias_t[:, grp:grp + 1]
            if eng == "dve":
                nc.vector.tensor_scalar(p_t[:, :width], p_t[:, :width],
                                        bslice, 0.0, op0=Alu.add, op1=Alu.max,
                                        accum_out=acc_t[:, grp:grp + 1])
            else:
                nc.scalar.activation(p_t[:, :width], p_t[:, :width], Relu,
                                     bias=bslice,
                                     accum_out=acc_t[:, grp:grp + 1])
            if grp == nbulk - 1:
                nc.sync.dma_start(acc_d[:, 0:nbulk], acc_t[:, 0:nbulk])

        # single suffix DMA for the last groups' accum columns
        nc.sync.dma_start(acc_d[:, nbulk:], acc_t[:, nbulk:])

    nc.compile()
    return nc


def _get_program():
    global _COMPILED
    if _COMPILED is None:
        _COMPILED = _build_program()
    return _COMPILED


def _core_blocks(k):
    """17 (row, col) upper-tri blocks for core k; the 2 diagonal first."""
    ra, rb = k, NBLK - 1 - k
    order = [(ra, ra), (rb, rb)]
    order += [(ra, c) for c in range(ra + 1, NBLK)]
    order += [(rb, c) for c in range(rb + 1, NBLK)]
    assert len(order) == NBLOCKS
    return order


def kernel(inputs: np.ndarray, target: np.ndarray) -> np.ndarray:
    global LAST_RESULTS
    x = np.asarray(inputs, dtype=np.float32)
    t = np.asarray(target).astype(np.int64)
    assert x.shape == (N, D) and t.shape == (N,)

    perm = np.argsort(t, kind="stable")
    xs = x[perm]
    ts = t[perm]

    x8 = xs.astype(FP8)                       # [N, 256]
    x8f = x8.astype(np.float32)
    sq = (xs.astype(np.float64) ** 2).sum(axis=1).astype(np.float32)
    sq_mean = float(sq.mean())

    # device row bias: B_i = m/2 - sq_i/2 - mean(sq)/2  (fp32; the column
    # part of the true bias is replaced by its mean, validated below).
    Bv = (0.5 * MARGIN - 0.5 * sq - 0.5 * sq_mean).astype(np.float32)

    # packed operand [128, 2, N]: xop[p, s, n] = x8[n, s*128 + p]
    xop = np.ascontiguousarray(x8.T.reshape(2, P, N).transpose(1, 0, 2))

    in_maps = []
    for k in range(NCORES):
        order = _core_blocks(k)
        ops = np.empty((P, NSLOTS * 2, BLK), FP8)
        bias_tab = np.empty((P, NGROUPS), np.float32)
        for pos, (r, cb) in enumerate(order):
            rsl = slice(r * BLK, (r + 1) * BLK)
            csl = slice(cb * BLK, (cb + 1) * BLK)
            ls, rs = _pos_slots(pos)
            ops[:, 2 * ls:2 * ls + 2, :] = xop[:, :, rsl]
            if rs != ls:
                ops[:, 2 * rs:2 * rs + 2, :] = xop[:, :, csl]
        for grp, (parts, width, wgt) in enumerate(GROUPS):
            bv = np.full(P, -np.inf, np.float32)
            for pos, mi, off, w, pcol in parts:
                r = order[pos][0]
                rows = slice(r * BLK + mi * P, r * BLK + (mi + 1) * P)
                bv = np.maximum(bv, Bv[rows])
            bias_tab[:, grp] = bv
        in_maps.append({"ops": ops, "biast": bias_tab})

    nc = _get_program()
    res = bass_utils.run_bass_kernel_spmd(
        nc, in_maps, core_ids=list(range(NCORES)))
    LAST_RESULTS = res

    W_ALL = np.array([g[2] for g in GROUPS])
    total = 0.0
    for k in range(NCORES):
        acc = res.results[k]["acc"].astype(np.float64)
        total += (acc.sum(axis=0) * W_ALL).sum()

    # exact diagonal-entry term (device computed i==j inside the weight-2
    # packed-squares group, with that group's max-bias vector)
    g_ii = (x8f * x8f).sum(axis=1, dtype=np.float32)
    diag_term = 0.0
    for k in range(NCORES):
        order = _core_blocks(k)
        for sg in SQ_GRPS:
            parts, _w, wgt = GROUPS[sg]
            pb = np.full(P, -np.inf, np.float32)
            for pos, mi, off, w, pcol in parts:
                r = order[pos][0]
                rows = slice(r * BLK + mi * P, r * BLK + (mi + 1) * P)
                pb = np.maximum(pb, Bv[rows])
            for pos, mi, off, w, pcol in parts:
                r = order[pos][0]
                rows = slice(r * BLK + mi * P, r * BLK + (mi + 1) * P)
                a_ii = g_ii[rows] + pb
                diag_term += wgt * np.maximum(a_ii, 0.0).astype(np.float64).sum()

    # a-posteriori zero-certificate: relu contributions are nonnegative, so
    # any off-diagonal false positive (from the dropped per-column bias part)
    # makes the device total exceed the exactly-emulated diagonal term.
    # Real data clears the margin by >100 absolute per pair; if an
    # adversarial input ever tripped this, recompute the term on the host.
    if abs(total - diag_term) > 1.0 + 1e-4 * abs(diag_term):
        total = _full_host_relu_term(x8f, sq)
    else:
        total -= diag_term

    # exact same-class term in fp64: sum_{same, i != j} [d - relu(m - d)]
    sq64 = (xs.astype(np.float64) ** 2).sum(axis=1)
    nclasses = int(ts.max()) + 1
    counts = np.bincount(ts, minlength=nclasses)
    starts = np.concatenate([[0], np.cumsum(counts)])
    for cc in range(nclasses):
        lo, hi = starts[cc], starts[cc + 1]
        if hi - lo < 2:
            continue
        Xc = xs[lo:hi].astype(np.float64)
        sqc = sq64[lo:hi]
        dm = sqc[:, None] + sqc[None, :] - 2.0 * (Xc @ Xc.T)
        np.fill_diagonal(dm, np.nan)
        total += np.nansum(dm) - np.nansum(np.maximum(MARGIN - dm, 0.0))

    loss = total / (N * (N - 1.0) * 2.0)
    return np.float32(loss)


def _full_host_relu_term(x8f, sq):
    """Fallback: the exact quantity `total` represents
    (4 * sum_{i<j} relu(a_ij)), computed blockwise on the host in fp64."""
    tot = 0.0
    B = 512
    x64 = x8f.astype(np.float64)
    sq64 = sq.astype(np.float64)
    for r0 in range(0, N, B):
        for c0 in range(r0, N, B):
            G = x64[r0:r0 + B] @ x64[c0:c0 + B].T
            a = G + 0.5 * (MARGIN - sq64[r0:r0 + B, None]
                           - sq64[None, c0:c0 + B])
            if c0 == r0:
                ri = np.arange(r0, r0 + B)
                a[ri[:, None] <= ri[None, :]] = -1.0
            tot += 4.0 * np.maximum(a, 0.0).sum()
    return tot


# revision 35
# speedup vs baseline: 1.0217x; 1.0217x over previous
"""Contrastive pairwise-margin loss on 8 Trainium2 NeuronCores.

loss = sum_{i,j} [ R_ij * d_ij + (1-R_ij) * relu(0.5 - d_ij) ] / (N*(N-1)*2)
with d_ij = ||x_i - x_j||^2 and R_ij = [t_i == t_j].

Decomposition (host rows sorted by class):
  loss_sum = sum_{i!=j} relu(m - d_ij)  +  sum_{same-class, i!=j} [d - relu(m-d)]
The second term is exact fp64 on the host (O(sum n_c^2 * D), tiny). The device
computes the uniform all-pairs relu sum over the upper-triangle blocks:
  relu(m - d_ij) = 2 * relu(a_ij),  a_ij = g_ij + (m - sq_i - sq_j)/2
Per [128,<=512] tile: fp8 DoubleRow gram matmuls accumulate g into PSUM; the
row part of the bias, B_i = m/2 - sq_i/2 - mean(sq)/2, is applied by the
relu+row-sum op itself (per-partition scalar operand) — no extra matmuls.
The column part (sq_j - mean(sq))/2 is dropped on device; since relu terms
are nonnegative, any false positive would show up as the device total
exceeding the exactly-emulated diagonal term, which the host checks, falling
back to an exact fp64 recompute if it ever failed (it cannot for data with
pairwise distances >> margin).  relu+row-sum is one fused op per PSUM group,
split between VectorE (tensor_scalar add-bias/max-0 + accum) and ScalarE
(activation Relu + bias + accum_out) by a static cost balance.  Groups mixing
row-chunks (diagonal strict-upper slices, packed 128x128 diagonal squares)
use the elementwise max of the chunks' bias vectors.  Host: everything x4
(2 from relu(2a)=2relu(a), 2 from block symmetry), x2 for the diagonal
squares (ordered pairs counted both ways), minus the exactly-emulated device
diagonal entries, plus the exact same-class term.
"""

import os
import sys

for _p in ("/opt/trn_rl_repo", "/root/.axon_site/_ro/trn_rl_repo"):
    if os.path.isdir(_p) and _p not in sys.path:
        sys.path.insert(0, _p)

from contextlib import ExitStack

import ml_dtypes
import numpy as np

import concourse.bass as bass  # noqa: F401
import concourse.mybir as mybir
from concourse import bacc, bass_utils
from concourse.tile import TileContext

FP8 = ml_dtypes.float8_e4m3
MARGIN = 0.5
N = 8192
D = 256
P = 128
BLK = 512
NBLK = N // BLK        # 16 stripes
NCORES = 8
NBLOCKS = 17           # blocks per core (2 diag + 15 off-diag)
NWARM = 6              # PE p-state warm-up matmuls

# operand slots: 0,1 = diag blocks (lhs==rhs); off-diag block i (pos 2..16)
# has lhs slot 2+2*(i-2), rhs slot 3+2*(i-2)  -> 2 + 15*2 = 32 slots
NSLOTS = 2 + 15 * 2

# DMA chunking in slot units (first chunks small so compute starts early)
SLOT_CHUNKS = [1, 1, 2, 2, 4, 6, 8, 8]
assert sum(SLOT_CHUNKS) == NSLOTS

# group table: (parts, width, wgt); parts = (block_pos, mi, rhs_off, w, pcol)
# diag block a: strict-upper slices split into TWO groups (one per engine, so
# both engines start on slot 0); diag block b merged into one 768 group; the
# eight 128x128 diagonal sub-squares of both diag blocks packed into one
# 1024-wide group (weight 2).
GROUPS = []
GROUPS.append(([(0, 0, 128, 384, 0)], 384, 4.0))
GROUPS.append(([(0, 1, 256, 256, 0), (0, 2, 384, 128, 256)], 384, 4.0))
GROUPS.append(([(0, mi, mi * P, P, mi * P) for mi in range(4)], 512, 2.0))
GROUPS.append(([(1, 0, 128, 384, 0), (1, 1, 256, 256, 384),
                (1, 2, 384, 128, 640)], 768, 4.0))
GROUPS.append(([(1, mi, mi * P, P, mi * P) for mi in range(4)], 512, 2.0))
for _pos in range(2, NBLOCKS):
    GROUPS.append(([(_pos, 0, 0, BLK, 0), (_pos, 1, 0, BLK, BLK)], 1024, 4.0))
    GROUPS.append(([(_pos, 2, 0, BLK, 0), (_pos, 3, 0, BLK, BLK)], 1024, 4.0))
NGROUPS = len(GROUPS)
SQ_GRPS = [i for i, g in enumerate(GROUPS) if g[2] == 2.0]

# static engine split: strict alternation is globally balanced (widths are
# symmetric) and locally smooth, so neither engine starves on the PSUM ring
def _op_cost(width, eng):
    if eng == "dve":
        return (width + 120) * 1.0417
    return (width + 172) * 0.8333 + 187

def _assign_engines():
    # head: the slot0-only half of the diagonal squares (g2) keeps DVE busy
    # across the slot-1 arrival seam; off-diagonals alternate DVE-first
    head = ["dve", "act", "dve", "act", "dve"]
    tail = ["dve" if i % 2 == 0 else "act" for i in range(len(GROUPS) - 5)]
    return head + tail

ENGINES = _assign_engines()
DVE_COLS = [i for i, e in enumerate(ENGINES) if e == "dve"]
ACC_COL = {}
for _i, _g in enumerate(DVE_COLS):
    ACC_COL[_g] = ("dve", _i)
_ACT_COLS = [i for i, e in enumerate(ENGINES) if e == "act"]
for _i, _g in enumerate(_ACT_COLS):
    ACC_COL[_g] = ("act", _i)
N_DVE = len(DVE_COLS)
N_ACT = len(_ACT_COLS)
W_DVE = np.array([GROUPS[g][2] for g in DVE_COLS])
W_ACT = np.array([GROUPS[g][2] for g in _ACT_COLS])

_CHUNK_OF_SLOT = {}
_c0 = 0
for _g, _n in enumerate(SLOT_CHUNKS):
    for _s in range(_c0, _c0 + _n):
        _CHUNK_OF_SLOT[_s] = (_g, _s - _c0)
    _c0 += _n

def _pos_slots(pos):
    """(lhs_slot, rhs_slot) for block position."""
    if pos < 2:
        return pos, pos
    return 2 + 2 * (pos - 2), 3 + 2 * (pos - 2)

_COMPILED = None
LAST_RESULTS = None


def _build_program():
    nc = bacc.Bacc("TRN2", target_bir_lowering=False, debug=False,
                   num_devices=NCORES)
    f8 = mybir.dt.float8e4
    f32 = mybir.dt.float32
    Alu = mybir.AluOpType
    Relu = mybir.ActivationFunctionType.Relu

    ops_d = nc.dram_tensor("ops", [P, NSLOTS * 2, BLK], f8,
                           kind="ExternalInput")
    bias_d = nc.dram_tensor("biast", [P, NGROUPS], f32, kind="ExternalInput")
    acc_d = nc.dram_tensor("acc", [P, NGROUPS], f32, kind="ExternalOutput")

    with TileContext(nc) as tc, ExitStack() as ctx:
        sb = ctx.enter_context(tc.tile_pool(name="sb", bufs=1))
        wpool = ctx.enter_context(tc.tile_pool(name="wpool", bufs=2))
        pp = ctx.enter_context(tc.tile_pool(name="pp", bufs=4, space="PSUM"))

        bias_t = sb.tile([P, NGROUPS], f32)
        acc_t = sb.tile([P, NGROUPS], f32)

        # input DMAs first, spread over three descriptor-generator paths so
        # they pipeline: slot chunk 0 via gpsimd SWDGE (cheapest first-byte),
        # the first bias columns via the DVE-issued HWDGE, the rest via sync.
        slots_g = []
        c0 = 0
        for g, nb in enumerate(SLOT_CHUNKS):
            bt = sb.tile([P, nb * 2, BLK], f8, tag=f"ops{g}")
            if g == 0:
                nc.sync.dma_start(bt[:], ops_d[:, 0:2, :])
                nc.gpsimd.dma_start(bias_t[:], bias_d[:])
            else:
                nc.sync.dma_start(bt[:], ops_d[:, c0 * 2:(c0 + nb) * 2, :])
            slots_g.append((c0, bt))
            c0 += nb

        # warm the ACT table set while DMAs ramp (hides LoadActFuncSet)
        warm = wpool.tile([P, 1], f32, tag="warm")
        nc.gpsimd.memset(warm[:], 0.0)
        nc.scalar.activation(warm[:], warm[:], Relu)

        # PE p-state warm-up: chain of cheap matmuls on a zeroed operand
        wop = wpool.tile([P, 2, 256], f8, tag="wop")
        nc.gpsimd.memset(wop[:], 0.0)
        wp = pp.tile([P, 2 * BLK], f32, tag="p")
        DR = mybir.MatmulPerfMode.DoubleRow
        for _ in range(NWARM):
            nc.tensor.matmul(wp[:, 0:256], wop[:, :, 0:P], wop[:],
                             start=True, stop=True, perf_mode=DR)

        def slot_ap(s):
            g, off = _CHUNK_OF_SLOT[s]
            bt = slots_g[g][1]
            return bt[:, off * 2:off * 2 + 2, :]   # [128, 2, 512]

        nbulk = NGROUPS - 8
        for grp, (parts, width, _wgt) in enumerate(GROUPS):
            p_t = pp.tile([P, 2 * BLK], f32, tag="p")
            for pos, mi, off, w, pcol in parts:
                ls, rs = _pos_slots(pos)
                lhs3 = slot_ap(ls)
                rhs3 = slot_ap(rs)
                sl = slice(pcol, pcol + w)
                nc.tensor.matmul(p_t[:, sl],
                                 lhs3[:, :, mi * P:(mi + 1) * P],
                                 rhs3[:, :, off:off + w],
                                 start=True, stop=True, perf_mode=DR)
            eng = ENGINES[grp]
            bslice = bias_t[:, grp:grp + 1]
            if eng == "dve":
                nc.vector.tensor_scalar(p_t[:, :width], p_t[:, :width],
                                        bslice, 0.0, op0=Alu.add, op1=Alu.max,
                                        accum_out=acc_t[:, grp:grp + 1])
            else:
                nc.scalar.activation(p_t[:, :width], p_t[:, :width], Relu,
                                     bias=bslice,
                                     accum_out=acc_t[:, grp:grp + 1])
            if grp == nbulk - 1:
                nc.sync.dma_start(acc_d[:, 0:nbulk], acc_t[:, 0:nbulk])

        # single suffix DMA for the last groups' accum columns
        nc.sync.dma_start(acc_d[:, nbulk:], acc_t[:, nbulk:])

    nc.compile()
    return nc


def _get_program():
    global _COMPILED
    if _COMPILED is None:
        _COMPILED = _build_program()
    return _COMPILED


def _core_blocks(k):
    """17 (row, col) upper-tri blocks for core k; the 2 diagonal first."""
    ra, rb = k, NBLK - 1 - k
    order = [(ra, ra), (rb, rb)]
    order += [(ra, c) for c in range(ra + 1, NBLK)]
    order += [(rb, c) for c in range(rb + 1, NBLK)]
    assert len(order) == NBLOCKS
    return order


def kernel(inputs: np.ndarray, target: np.ndarray) -> np.ndarray:
    global LAST_RESULTS
    x = np.asarray(inputs, dtype=np.float32)
    t = np.asarray(target).astype(np.int64)
    assert x.shape == (N, D) and t.shape == (N,)

    perm = np.argsort(t, kind="stable")
    xs = x[perm]
    ts = t[perm]

    x8 = xs.astype(FP8)                       # [N, 256]
    x8f = x8.astype(np.float32)
    sq = (xs.astype(np.float64) ** 2).sum(axis=1).astype(np.float32)
    sq_mean = float(sq.mean())

    # device row bias: B_i = m/2 - sq_i/2 - mean(sq)/2  (fp32; the column
    # part of the true bias is replaced by its mean, validated below).
    Bv = (0.5 * MARGIN - 0.5 * sq - 0.5 * sq_mean).astype(np.float32)

    # packed operand [128, 2, N]: xop[p, s, n] = x8[n, s*128 + p]
    xop = np.ascontiguousarray(x8.T.reshape(2, P, N).transpose(1, 0, 2))

    in_maps = []
    for k in range(NCORES):
        order = _core_blocks(k)
        ops = np.empty((P, NSLOTS * 2, BLK), FP8)
        bias_tab = np.empty((P, NGROUPS), np.float32)
        for pos, (r, cb) in enumerate(order):
            rsl = slice(r * BLK, (r + 1) * BLK)
            csl = slice(cb * BLK, (cb + 1) * BLK)
            ls, rs = _pos_slots(pos)
            ops[:, 2 * ls:2 * ls + 2, :] = xop[:, :, rsl]
            if rs != ls:
                ops[:, 2 * rs:2 * rs + 2, :] = xop[:, :, csl]
        for grp, (parts, width, wgt) in enumerate(GROUPS):
            bv = np.full(P, -np.inf, np.float32)
            for pos, mi, off, w, pcol in parts:
                r = order[pos][0]
                rows = slice(r * BLK + mi * P, r * BLK + (mi + 1) * P)
                bv = np.maximum(bv, Bv[rows])
            bias_tab[:, grp] = bv
        in_maps.append({"ops": ops, "biast": bias_tab})

    nc = _get_program()
    res = bass_utils.run_bass_kernel_spmd(
        nc, in_maps, core_ids=list(range(NCORES)))
    LAST_RESULTS = res

    W_ALL = np.array([g[2] for g in GROUPS])
    total = 0.0
    for k in range(NCORES):
        acc = res.results[k]["acc"].astype(np.float64)
        total += (acc.sum(axis=0) * W_ALL).sum()

    # exact diagonal-entry term (device computed i==j inside the weight-2
    # packed-squares group, with that group's max-bias vector)
    g_ii = (x8f * x8f).sum(axis=1, dtype=np.float32)
    diag_term = 0.0
    for k in range(NCORES):
        order = _core_blocks(k)
        for sg in SQ_GRPS:
            parts, _w, wgt = GROUPS[sg]
            pb = np.full(P, -np.inf, np.float32)
            for pos, mi, off, w, pcol in parts:
                r = order[pos][0]
                rows = slice(r * BLK + mi * P, r * BLK + (mi + 1) * P)
                pb = np.maximum(pb, Bv[rows])
            for pos, mi, off, w, pcol in parts:
                r = order[pos][0]
                rows = slice(r * BLK + mi * P, r * BLK + (mi + 1) * P)
                a_ii = g_ii[rows] + pb
                diag_term += wgt * np.maximum(a_ii, 0.0).astype(np.float64).sum()

    # a-posteriori zero-certificate: relu contributions are nonnegative, so
    # any off-diagonal false positive (from the dropped per-column bias part)
    # makes the device total exceed the exactly-emulated diagonal term.
    # Real data clears the margin by >100 absolute per pair; if an
    # adversarial input ever tripped this, recompute the term on the host.
    if abs(total - diag_term) > 1.0 + 1e-4 * abs(diag_term):
        total = _full_host_relu_term(x8f, sq)
    else:
        total -= diag_term

    # exact same-class term in fp64: sum_{same, i != j} [d - relu(m - d)]
    sq64 = (xs.astype(np.float64) ** 2).sum(axis=1)
    nclasses = int(ts.max()) + 1
    counts = np.bincount(ts, minlength=nclasses)
    starts = np.concatenate([[0], np.cumsum(counts)])
    for cc in range(nclasses):
        lo, hi = starts[cc], starts[cc + 1]
        if hi - lo < 2:
            continue
        Xc = xs[lo:hi].astype(np.float64)
        sqc = sq64[lo:hi]
        dm = sqc[:, None] + sqc[None, :] - 2.0 * (Xc @ Xc.T)
        np.fill_diagonal(dm, np.nan)
        total += np.nansum(dm) - np.nansum(np.maximum(MARGIN - dm, 0.0))

    loss = total / (N * (N - 1.0) * 2.0)
    return np.float32(loss)


def _full_host_relu_term(x8f, sq):
    """Fallback: the exact quantity `total` represents
    (4 * sum_{i<j} relu(a_ij)), computed blockwise on the host in fp64."""
    tot = 0.0
    B = 512
    x64 = x8f.astype(np.float64)
    sq64 = sq.astype(np.float64)
    for r0 in range(0, N, B):
        for c0 in range(r0, N, B):
            G = x64[r0:r0 + B] @ x64[c0:c0 + B].T
            a = G + 0.5 * (MARGIN - sq64[r0:r0 + B, None]
                           - sq64[None, c0:c0 + B])
            if c0 == r0:
                ri = np.arange(r0, r0 + B)
                a[ri[:, None] <= ri[None, :]] = -1.0
            tot += 4.0 * np.maximum(a, 0.0).sum()
    return tot
